# revision 4
# baseline (speedup 1.0000x reference)
"""HEPT Gaussian-kernel attention on 8 trn2 NeuronCores (Bass/Tile).

Problem: H=8 heads, B=4 batch, N=2048, dc=8, d=64.
  kernel(q,k) = exp(-0.5*|q-k|^2); attn = kernel / (rowsum(kernel)+EPS);
  out = attn @ V, per independent (b,h) slice.

Sharding: one head per core (8 heads / 8 cores); each core computes its
head's 4 batch slices. No cross-core communication.

Device algorithm per (b) slice, keys on partitions:
  exponent S'[key,query] = sum_c Ka[c,key]*Qa[c,query] over 11 augmented
  bf16 rows: rows 0-7 Q|K, row 8 (1 | k2hi), row 9 (1 | k2lo),
  row 10 (-q^2/2 | 1); k2hi+k2lo is an accurate bf16 split of -k^2/2
  computed on device (squares on DVE, column sums via 1-col fp32 matmul).
  P^T = exp(S') on ScalarE in [128,1024] tiles (the throughput bound).
  out^T[65,q] += V'[key,:]^T @ P^T with V' = [V | 1], so row 64
  accumulates the denominator; EPS is seeded by a rank-1 start-matmul.
  Normalize with reciprocal_approx_fast + a rank-1 fp32 broadcast matmul,
  DMA out^T [64, N]; the host transposes back.

Masking: host prep moves masked queries to +1e4 and masked keys to -1e4,
which reproduces masked semantics exactly (masked pairs underflow to
exp(-huge)=0; fully-masked query rows give 0/(0+EPS)=0).
"""

import sys

for _p in ("/root/.axon_site/_ro/trn_rl_repo", "/opt/trn_rl_repo"):
    if _p not in sys.path:
        sys.path.append(_p)

import numpy as np
import ml_dtypes

import concourse.mybir as mybir
import concourse.tile as tile
from concourse import bacc
from concourse.bass_utils import run_bass_kernel_spmd

H, B, N, DC, D = 8, 4, 2048, 8, 64
EPS = 2.0 ** -4
N_CORES = 8
NKT = N // 128          # 16 key tiles per slice
NQB = N // 512          # 4 query blocks per slice
VC = D + 1              # V columns + ones column
AUG = 11                # augmented contraction rows
BF16 = mybir.dt.bfloat16
FP32 = mybir.dt.float32
Exp = mybir.ActivationFunctionType.Exp


def build_kernel(repeat: int = 1) -> bacc.Bacc:
    nc = bacc.Bacc()
    qT = nc.declare_dram_parameter("qT", [B, AUG, N], BF16, isOutput=False)
    kT = nc.declare_dram_parameter("kT", [B, AUG, N], BF16, isOutput=False)
    v = nc.declare_dram_parameter("v", [B, N, D], BF16, isOutput=False)
    ot = nc.declare_dram_parameter("ot", [B, D, N], FP32, isOutput=True)

    with tile.TileContext(nc) as tc:
        with (
            tc.tile_pool(name="consts", bufs=1) as consts,
            tc.tile_pool(name="aug", bufs=2) as aug_pool,
            tc.tile_pool(name="prep", bufs=1) as prep_pool,
            tc.tile_pool(name="vt", bufs=2) as vt_pool,
            tc.tile_pool(name="pst", bufs=3) as pst_pool,
            tc.tile_pool(name="outs", bufs=2) as outs_pool,
            tc.tile_pool(name="stp", bufs=2, space="PSUM") as stp_pool,
            tc.tile_pool(name="opsum", bufs=1, space="PSUM") as opsum_pool,
        ):
            # constants
            mhalf = consts.tile([DC, 1], FP32, name="mhalf")
            nc.vector.memset(mhalf, -0.5)
            epsv = consts.tile([1, VC], BF16, name="epsv")
            nc.vector.memset(epsv, 0.0)
            nc.vector.memset(epsv[:, D:VC], EPS)
            ones_q = consts.tile([1, 512], BF16, name="ones_q")
            nc.vector.memset(ones_q, 1.0)
            ones64f = consts.tile([1, D], FP32, name="ones64f")
            nc.vector.memset(ones64f, 1.0)

            def body(_iv=None):
                for b in range(B):
                    # ---- per-slice prep: augmented Qa / Ka ----
                    qa = aug_pool.tile([AUG, N], BF16, name=f"qa{b}", tag="qa")
                    ka = aug_pool.tile([AUG, N], BF16, name=f"ka{b}", tag="ka")
                    nc.sync.dma_start(out=qa, in_=qT[b])
                    nc.sync.dma_start(out=ka, in_=kT[b])
                    sqq = prep_pool.tile([DC, N], FP32, name=f"sqq{b}", tag="sqq")
                    sqk = prep_pool.tile([DC, N], FP32, name=f"sqk{b}", tag="sqk")
                    nc.vector.tensor_mul(out=sqq, in0=qa[0:DC, :], in1=qa[0:DC, :])
                    nc.vector.tensor_mul(out=sqk, in0=ka[0:DC, :], in1=ka[0:DC, :])
                    # -0.5 * colsum(x^2) via 1-column fp32 matmuls; bf16 rows
                    # staged at partition 0, then DMA'd into their aug rows.
                    scr_q2 = prep_pool.tile([1, N], BF16, name=f"scq{b}", tag="scq")
                    scr_hi = prep_pool.tile([1, N], BF16, name=f"sch{b}", tag="sch")
                    scr_lo = prep_pool.tile([1, N], BF16, name=f"scl{b}", tag="scl")
                    for t, sq in enumerate((sqq, sqk)):
                        for j in range(NQB):
                            js = slice(j * 512, (j + 1) * 512)
                            pp = stp_pool.tile([1, 512], FP32,
                                               name=f"pp{b}_{t}_{j}", tag="stp")
                            nc.tensor.matmul(pp, mhalf, sq[:, js],
                                             start=True, stop=True)
                            if t == 0:
                                # q^2: plain bf16 (its error cancels in the ratio)
                                nc.vector.tensor_copy(out=scr_q2[:, js], in_=pp)
                            else:
                                # k^2: accurate hi+lo bf16 split
                                nc.vector.tensor_copy(out=scr_hi[:, js], in_=pp)
                                nc.vector.tensor_sub(out=scr_lo[:, js],
                                                     in0=pp, in1=scr_hi[:, js])
                    nc.sync.dma_start(out=qa[10:11, :], in_=scr_q2)
                    nc.sync.dma_start(out=ka[8:9, :], in_=scr_hi)
                    nc.sync.dma_start(out=ka[9:10, :], in_=scr_lo)

                    # ---- V' = [V | 1] as 16 key tiles on partitions ----
                    vt = vt_pool.tile([128, NKT, VC], BF16, name=f"vt{b}", tag="vt")
                    nc.vector.memset(vt[:, :, D:VC], 1.0)
                    vsrc = v[b].rearrange("(t p) d -> p t d", p=128)
                    nc.sync.dma_start(out=vt[:, :, 0:D], in_=vsrc)

                    # ---- output accumulators: EPS seed via rank-1 matmul ----
                    ops = []
                    for qb in range(NQB):
                        op = opsum_pool.tile([VC, 512], FP32,
                                             name=f"op{b}_{qb}", tag=f"op{qb}")
                        nc.tensor.matmul(op, epsv, ones_q, start=True, stop=False)
                        ops.append(op)

                    # ---- main loop: kt outer, query-pair inner ----
                    for kt in range(NKT):
                        ka_t = ka[:, kt * 128:(kt + 1) * 128]
                        vt_t = vt[:, kt, :]
                        for qp in range(NQB // 2):
                            stp = stp_pool.tile([128, 1024], FP32,
                                                name=f"stp{b}_{kt}_{qp}", tag="stp")
                            for j in range(2):
                                nc.tensor.matmul(
                                    stp[:, j * 512:(j + 1) * 512], ka_t,
                                    qa[:, (qp * 2 + j) * 512:(qp * 2 + j + 1) * 512],
                                    start=True, stop=True)
                            pst = pst_pool.tile([128, 1024], BF16,
                                                name=f"pst{b}_{kt}_{qp}", tag="pst")
                            nc.scalar.activation(out=pst, in_=stp, func=Exp)
                            for j in range(2):
                                nc.tensor.matmul(
                                    ops[qp * 2 + j], vt_t,
                                    pst[:, j * 512:(j + 1) * 512],
                                    start=False, stop=(kt == NKT - 1))

                    # ---- normalize and store ----
                    for qb in range(NQB):
                        dn = outs_pool.tile([1, 512], FP32,
                                            name=f"dn{b}_{qb}", tag="dn")
                        nc.vector.tensor_copy(out=dn, in_=ops[qb][D:VC, :])
                        r_row = outs_pool.tile([1, 512], FP32,
                                               name=f"r{b}_{qb}", tag="r_row")
                        nc.vector.reciprocal_approx_fast(r_row, dn)
                        rps = stp_pool.tile([D, 512], FP32,
                                            name=f"rps{b}_{qb}", tag="stp")
                        nc.tensor.matmul(rps, ones64f, r_row, start=True, stop=True)
                        rsb = outs_pool.tile([D, 512], FP32,
                                             name=f"rsb{b}_{qb}", tag="rsb")
                        nc.vector.tensor_copy(out=rsb, in_=rps)
                        osb = outs_pool.tile([D, 512], FP32,
                                             name=f"osb{b}_{qb}", tag="osb")
                        nc.vector.tensor_mul(out=osb, in0=ops[qb][0:D, :], in1=rsb)
                        nc.sync.dma_start(
                            out=ot[b, :, qb * 512:(qb + 1) * 512], in_=osb)

            if repeat == 1:
                body()
            else:
                with tc.For_i(0, repeat, 1) as iv:
                    body(iv)
    nc.compile()
    return nc


_RUNNER = {}


def _get_runner(repeat: int = 1):
    if repeat not in _RUNNER:
        _RUNNER[repeat] = build_kernel(repeat)
    return _RUNNER[repeat]


def prep_inputs(query, key, value, padding_mask):
    """Host-side shard/layout prep: per-head slices, transposes, bf16 casts,
    aug constant rows, mask folded into coordinates."""
    bf16 = ml_dtypes.bfloat16
    q = np.asarray(query, np.float32).reshape(H, B, N, DC)
    k = np.asarray(key, np.float32).reshape(H, B, N, DC)
    val = np.asarray(value, np.float32).reshape(H, B, N, D)
    m = np.asarray(padding_mask, bool)
    if not m.all():
        q, k = q.copy(), k.copy()
        q[:, ~m, :] = 1e4
        k[:, ~m, :] = -1e4
    qa = np.zeros((H, B, AUG, N), bf16)
    ka = np.zeros((H, B, AUG, N), bf16)
    qa[:, :, 0:DC, :] = q.transpose(0, 1, 3, 2).astype(bf16)
    ka[:, :, 0:DC, :] = k.transpose(0, 1, 3, 2).astype(bf16)
    qa[:, :, DC:DC + 2, :] = 1.0   # rows 8,9 pair k2hi/k2lo
    ka[:, :, DC + 2, :] = 1.0      # row 10 pairs -q^2/2
    vb = val.astype(bf16)
    return [{"qT": qa[h], "kT": ka[h], "v": vb[h]} for h in range(H)]


def kernel(query, key, value, padding_mask):
    nc = _get_runner()
    in_maps = prep_inputs(query, key, value, padding_mask)
    res = run_bass_kernel_spmd(nc, in_maps, list(range(N_CORES)))
    out = np.empty((H, B * N, D), np.float32)
    for h in range(N_CORES):
        o = res.results[h]["ot"]  # (B, D, N)
        out[h] = o.transpose(0, 2, 1).reshape(B * N, D)
    return out
